# revision 14
# baseline (speedup 1.0000x reference)
"""Trainium2 Bass kernel for nn_LLaVA_CLIP (loss_fn).

Math (exact reformulations of the reference):
- row_norm(x * c) == row_norm(x) for positive per-row scale c, so the
  exp(ls_*) factors vanish, sim_* == a_*, out_logits == a_total, and the
  row_norm of desc cancels inside row_norm(desc @ txt).
- quick_gelu(x) = silu(1.702 x)/1.702; silu(z) = 0.5 z (1 + tanh(z/2));
  constants fold into the next matmul's weights (W2/2).
- Both LayerNorms fold into the adjacent matmuls:
    e1_1 = rstd0 (.) [s0 @ (diag(w0ln) W1.T) + (-mu0) x cmu + irstd0 x cb1]
      cmu = w0ln @ W1.T, cb1 = b1 + b0ln @ W1.T, irstd0 = sqrt(var0+eps)
    S2 = s1 @ (diag(w1ln) txt) + (-mu1) x ct1 + irstd1 x ct2
      (the rstd1 row scale cancels in row_norm)
Sharding: batch 65536 -> 8 cores x 8192 rows, weights replicated. MLP runs
feature-major ([feat, batch] tiles); sims/logits run batch-major per 128 rows.
"""
import sys

sys.path.insert(0, "/opt/trn_rl_repo")
if "/root/.axon_site" not in sys.path:
    sys.path.insert(0, "/root/.axon_site")

import numpy as np

import concourse.bass as bass
import concourse.mybir as mybir
from concourse import tile
from concourse.bass_utils import run_bass_kernel_spmd

AF = mybir.ActivationFunctionType
OP = mybir.AluOpType
AX = mybir.AxisListType
f32 = mybir.dt.float32
f32r = mybir.dt.float32r

B, D, H, C = 65536, 512, 1024, 1000
NCORES = 8
ROWS = B // NCORES          # 8192 rows per core
BT = 512                    # batch columns per feature-major tile
NSUB = BT // 128            # batch-major subtiles per tile
KD = D // 128               # 4
KH = H // 128               # 8
NCOLS = ROWS // 128         # 64 staging columns
BIG = float(2 ** 20)
LN_EPS = 1e-5
CGRPS = [(0, 512), (512, 488)]

_CACHE = {}
DEBUG_T = False


def _split_waits(nc, cap=1):
    """This walrus accepts only ~1 sync-wait command per instruction; hoist
    extra waits onto same-engine EventSemaphore carriers (program order on the
    engine preserves semantics)."""
    for fn in nc.m.functions:
        for blk in fn.blocks:
            out = []
            for inst in blk.instructions:
                si = inst.sync_info
                if si is not None and si.on_wait and len(si.on_wait) > cap:
                    waits = list(si.on_wait)
                    keep, extra = waits[:cap], waits[cap:]
                    for i in range(0, len(extra), cap):
                        out.append(
                            mybir.InstEventSemaphore(
                                name=f"waitsplit-{nc.next_id()}",
                                engine=inst.engine,
                                sync_info=mybir.SyncInfo(
                                    on_wait=extra[i : i + cap], on_update=[]
                                ),
                                bass_nofuse=True,
                            )
                        )
                    inst.sync_info = mybir.SyncInfo(
                        on_wait=keep, on_update=list(si.on_update)
                    )
                out.append(inst)
            blk.instructions[:] = out


def build(ntiles, has_cb1, has_ct2, has_txw):
    nc = bass.Bass("TRN2", target_bir_lowering=False, debug=False)
    dt = nc.dram_tensor

    xT_d = dt("xT", [KD, 128, ROWS], f32, kind="ExternalInput")
    imT_d = dt("imT", [KD, 128, ROWS], f32, kind="ExternalInput")
    tgt_d = dt("tgtm", [128, NCOLS], f32, kind="ExternalInput")   # target-BIG
    A0_d = dt("A0", [128, KD, H], f32, kind="ExternalInput")
    W20_d = dt("W20", [128, KH, H], f32, kind="ExternalInput")
    W1f_d = dt("W1f", [128, KH, D], f32, kind="ExternalInput")
    W21_d = dt("W21", [128, KD, D], f32, kind="ExternalInput")
    txt_d = dt("txt4", [128, KD, C], f32, kind="ExternalInput")
    txw_d = dt("txw4", [128, KD, C], f32, kind="ExternalInput") if has_txw else None
    cmu_d = dt("cmu", [1, D], f32, kind="ExternalInput")
    cb1_d = dt("cb1", [1, D], f32, kind="ExternalInput")
    ct1_d = dt("ct1", [1, C], f32, kind="ExternalInput")
    ct2_d = dt("ct2", [1, C], f32, kind="ExternalInput")
    b0_d = dt("b0c", [128, KH], f32, kind="ExternalInput")
    b20_d = dt("b20c", [128, KH], f32, kind="ExternalInput")
    b21_d = dt("b21c", [128, KD], f32, kind="ExternalInput")
    iota_d = dt("iotam", [128, C], f32, kind="ExternalInput")     # iota-BIG
    lnp_d = dt("lnp", [128, 1], f32, kind="ExternalInput")        # ln(p)
    ln1p_d = dt("ln1p", [128, 1], f32, kind="ExternalInput")      # ln(1-p)
    nlt_d = dt("nlt", [128, 1], f32, kind="ExternalInput")        # -ln(temp)
    nll_d = dt("nll", [128, NCOLS], f32, kind="ExternalOutput")
    amx_d = dt("amx", [128, NCOLS], f32, kind="ExternalOutput")
    dbg_d = dt("dbgT", [128, C], f32, kind="ExternalOutput") if DEBUG_T else None
    if DEBUG_T:
        dbge1_d = dt("dbge1", [128, BT], f32, kind="ExternalOutput")
        dbgg_d = dt("dbgg", [128, BT], f32, kind="ExternalOutput")
        dbgs0_d = dt("dbgs0", [128, BT], f32, kind="ExternalOutput")
        dbge11_d = dt("dbge11", [128, BT], f32, kind="ExternalOutput")
        dbgs1_d = dt("dbgs1", [128, BT], f32, kind="ExternalOutput")
        dbgS2_d = dt("dbgS2", [128, C], f32, kind="ExternalOutput")
        dbgq1_d = dt("dbgq1", [128, 1], f32, kind="ExternalOutput")
        dbgq2_d = dt("dbgq2", [128, 1], f32, kind="ExternalOutput")
        dbgr1_d = dt("dbgr1", [128, 1], f32, kind="ExternalOutput")
        dbgr2_d = dt("dbgr2", [128, 1], f32, kind="ExternalOutput")

    with tile.TileContext(nc) as tc:
        with (
            tc.tile_pool(name="wpool", bufs=1) as wp,
            tc.tile_pool(name="w20ring", bufs=3) as wr,
            tc.tile_pool(name="inpool", bufs=2) as ip,
            tc.tile_pool(name="actpool", bufs=1) as ap_,
            tc.tile_pool(name="scpool", bufs=2) as sp,
            tc.tile_pool(name="psA", bufs=3, space="PSUM") as psA,
            tc.tile_pool(name="psStat", bufs=2, space="PSUM") as psT,
            tc.tile_pool(name="psSim", bufs=3, space="PSUM") as psS,
        ):
            # ---- resident constants -----------------------------------
            A0 = wp.tile([128, KD, H], f32, name="A0s")
            W1f = wp.tile([128, KH, D], f32, name="W1fs")
            W21 = wp.tile([128, KD, D], f32, name="W21s")
            txt = wp.tile([128, KD, C], f32, name="txts")
            txw = wp.tile([128, KD, C], f32, name="txws") if has_txw else txt
            cmu = wp.tile([1, D], f32, name="cmus")
            cb1 = wp.tile([1, D], f32, name="cb1s")
            ct1 = wp.tile([1, C], f32, name="ct1s")
            ct2 = wp.tile([1, C], f32, name="ct2s")
            b0 = wp.tile([128, KH], f32, name="b0s")
            b20 = wp.tile([128, KH], f32, name="b20s")
            b21 = wp.tile([128, KD], f32, name="b21s")
            iot = wp.tile([128, C], f32, name="iots")
            lnp = wp.tile([128, 1], f32, name="lnps")
            ln1p = wp.tile([128, 1], f32, name="ln1ps")
            nlt = wp.tile([128, 1], f32, name="nlts")
            tgt = wp.tile([128, NCOLS], f32, name="tgts")
            ones = wp.tile([128, 1], f32, name="oness")
            epsv = wp.tile([1, 1], f32, name="epsvs")
            onesr = wp.tile([1, 128], f32, name="onesrs")
            nll_st = wp.tile([128, NCOLS], f32, name="nllst")
            amx_st = wp.tile([128, NCOLS], f32, name="amxst")

            for t_, d_ in [
                (A0, A0_d), (W1f, W1f_d), (W21, W21_d), (txt, txt_d),
                *([(txw, txw_d)] if has_txw else []),
                (cmu, cmu_d), (cb1, cb1_d), (ct1, ct1_d),
                (ct2, ct2_d), (b0, b0_d), (b20, b20_d), (b21, b21_d),
                (iot, iota_d), (lnp, lnp_d), (ln1p, ln1p_d), (nlt, nlt_d),
                (tgt, tgt_d),
            ]:
                nc.sync.dma_start(t_[:], d_.ap())
            nc.gpsimd.memset(ones[:], 1.0)
            nc.gpsimd.memset(epsv[:], float(LN_EPS))
            nc.gpsimd.memset(onesr[:], 1.0)
            ones_r = wp.tile([128, 1], f32r, name="onesrr")
            onesr_r = wp.tile([1, 128], f32r, name="onesrrr")
            cmu_r = wp.tile([1, D], f32r, name="cmurr")
            ct1_r = wp.tile([1, C], f32r, name="ct1rr")
            nc.vector.tensor_copy(ones_r[:], ones[:])
            nc.vector.tensor_copy(onesr_r[:], onesr[:])
            nc.vector.tensor_copy(cmu_r[:], cmu[:])
            nc.vector.tensor_copy(ct1_r[:], ct1[:])

            for t in range(ntiles):
                bsl = bass.ts(t, BT)

                xT = ip.tile([128, KD, BT], f32, name=f"xT{t}", tag="xT")
                imT = ip.tile([128, KD, BT], f32, name=f"imT{t}", tag="imT")
                for k in range(KD):
                    nc.sync.dma_start(xT[:, k, :], xT_d.ap()[k, :, bsl])
                    nc.sync.dma_start(imT[:, k, :], imT_d.ap()[k, :, bsl])

                # ---- L0a: e1 = W0 @ xT + b0; g' = e1 (1 + tanh(.851 e1))
                e1s = ap_.tile([128, KH, BT], f32, name=f"e1s{t}", tag="e1s")
                gs = ap_.tile([128, KH, BT], f32, name=f"gs{t}", tag="gs")
                for m in range(KH):
                    pe1 = psA.tile([128, BT], f32, name=f"pe1_{t}_{m}", tag="psA")
                    for k in range(KD):
                        nc.tensor.matmul(
                            pe1[:], A0[:, k, bass.ts(m, 128)], xT[:, k, :],
                            start=(k == 0), stop=(k == KD - 1),
                        )
                    nc.scalar.activation(
                        e1s[:, m, :], pe1[:], AF.Identity,
                        bias=b0[:, m : m + 1], scale=1.0,
                    )
                    th = sp.tile([128, BT], f32, name=f"th{t}_{m}", tag="th", bufs=1)
                    nc.scalar.activation(
                        th[:], e1s[:, m, :], AF.Tanh, bias=0.0, scale=0.851
                    )
                    # g' = (th + 1) * e1
                    nc.vector.scalar_tensor_tensor(
                        gs[:, m, :], th[:], 1.0, e1s[:, m, :],
                        op0=OP.add, op1=OP.mult,
                    )

                if DEBUG_T and t == 0:
                    nc.sync.dma_start(dbge1_d.ap(), e1s[:, 0, :])
                    nc.sync.dma_start(dbgg_d.ap(), gs[:, 0, :])
                # ---- L0b: e2 = (W2_0/2) @ g'; s0 = e1+e2 (into e1s); stats
                pmu0 = psT.tile([1, BT], f32, name=f"pmu0{t}", tag="psT")
                pvr0 = psT.tile([1, BT], f32, name=f"pvr0{t}", tag="psT")
                for m in range(KH):
                    w20c = wr.tile([128, H], f32, name=f"w20_{t}_{m}", tag="w20")
                    nc.sync.dma_start(w20c[:], W20_d.ap()[:, m, :])
                    pe2 = psA.tile([128, BT], f32, name=f"pe2_{t}_{m}", tag="psA")
                    for k in range(KH):
                        nc.tensor.matmul(
                            pe2[:], w20c[:, bass.ts(k, 128)], gs[:, k, :],
                            start=(k == 0), stop=(k == KH - 1),
                        )
                    sq = sp.tile([128, BT], f32, name=f"sq0{t}_{m}", tag="sq")
                    sqr = sp.tile([128, BT], f32r, name=f"sqr0{t}_{m}", tag="sqr", bufs=1)
                    nc.scalar.activation(
                        sq[:], pe2[:], AF.Identity,
                        bias=b20[:, m : m + 1], scale=1.0,
                    )
                    # s0 overwrites e1s (e1 is dead after this)
                    nc.vector.tensor_tensor(
                        e1s[:, m, :], e1s[:, m, :], sq[:], op=OP.add
                    )
                    nc.scalar.activation(sqr[:], e1s[:, m, :], AF.Square)
                    s0r = sp.tile([128, BT], f32r, name=f"s0r{t}_{m}", tag="s0r", bufs=1)
                    nc.vector.tensor_copy(s0r[:], e1s[:, m, :])
                    nc.tensor.matmul(
                        pmu0[:], ones_r[:], s0r[:],
                        start=(m == 0), stop=(m == KH - 1),
                        skip_group_check=True,
                    )
                    nc.tensor.matmul(
                        pvr0[:], ones_r[:], sqr[:],
                        start=(m == 0), stop=(m == KH - 1),
                        skip_group_check=True,
                    )
                s0 = e1s
                if DEBUG_T and t == 0:
                    nc.sync.dma_start(dbgs0_d.ap(), s0[:, 0, :])

                # ---- LN0 scalars on [1, BT] ------------------------------
                nmu0 = sp.tile([1, BT], f32, name=f"nmu0{t}", tag="nmu0", bufs=1)
                var0 = sp.tile([1, BT], f32, name=f"var0{t}", tag="var0", bufs=1)
                rst0 = sp.tile([1, BT], f32, name=f"rst0{t}", tag="rst0", bufs=1)
                ist0 = sp.tile([1, BT], f32, name=f"ist0{t}", tag="ist0", bufs=1)
                nc.vector.tensor_scalar(nmu0[:], pmu0[:], -1.0 / H, None, op0=OP.mult)
                # var = msq - mu^2 = (pvr0/H) - nmu0^2
                nc.vector.tensor_tensor(rst0[:], nmu0[:], nmu0[:], op=OP.mult)
                nc.vector.scalar_tensor_tensor(
                    var0[:], pvr0[:], 1.0 / H, rst0[:], op0=OP.mult, op1=OP.subtract
                )
                nc.scalar.activation(var0[:], var0[:], AF.Ln, bias=epsv[:], scale=1.0)
                nc.scalar.activation(rst0[:], var0[:], AF.Exp, bias=0.0, scale=-0.5)
                nc.scalar.activation(ist0[:], var0[:], AF.Exp, bias=0.0, scale=0.5)
                rst0r = sp.tile([1, BT], f32r, name=f"rst0r{t}", tag="rst0r", bufs=1)
                nmu0r = sp.tile([1, BT], f32r, name=f"nmu0r{t}", tag="nmu0r", bufs=1)
                nc.vector.tensor_copy(rst0r[:], rst0[:])
                nc.vector.tensor_copy(nmu0r[:], nmu0[:])

                # ---- L1a: P = s0@W1f + (-mu0)x cmu + irstd0 x cb1;
                #      e1_1 = rstd0 (.) P; g2' = e1_1(1+tanh(.851 e1_1)) ----
                e11 = ap_.tile([128, KD, BT], f32, name=f"e11{t}", tag="e11")
                g2 = ap_.tile([128, KD, BT], f32, name=f"g2{t}", tag="gs")
                prb = psA.tile([128, BT], f32, name=f"prb{t}", tag="psA")
                nc.tensor.matmul(prb[:], onesr_r[:], rst0r[:], start=True, stop=True)
                rstb = sp.tile([128, BT], f32, name=f"rstb{t}", tag="rstb", bufs=1)
                nc.scalar.copy(rstb[:], prb[:])
                for m in range(KD):
                    pp = psA.tile([128, BT], f32, name=f"pp_{t}_{m}", tag="psA")
                    for k in range(KH):
                        nc.tensor.matmul(
                            pp[:], W1f[:, k, bass.ts(m, 128)], s0[:, k, :],
                            start=(k == 0), stop=False,
                        )
                    nc.tensor.matmul(
                        pp[:], cmu_r[:, bass.ts(m, 128)], nmu0r[:],
                        start=False, stop=not has_cb1,
                    )
                    if has_cb1:
                        nc.tensor.matmul(
                            pp[:], cb1[:, bass.ts(m, 128)], ist0[:],
                            start=False, stop=True,
                        )
                    nc.vector.tensor_tensor(e11[:, m, :], pp[:], rstb[:], op=OP.mult)
                    th2 = sp.tile([128, BT], f32, name=f"th2{t}_{m}", tag="th", bufs=1)
                    nc.scalar.activation(
                        th2[:], e11[:, m, :], AF.Tanh, bias=0.0, scale=0.851
                    )
                    nc.vector.scalar_tensor_tensor(
                        g2[:, m, :], th2[:], 1.0, e11[:, m, :],
                        op0=OP.add, op1=OP.mult,
                    )

                if DEBUG_T and t == 0:
                    nc.sync.dma_start(dbge11_d.ap(), e11[:, 0, :])
                # ---- L1b: e2_1 = (W2_1/2) @ g2'; s1 = e1_1+e2_1; stats ---
                pmu1 = psT.tile([1, BT], f32, name=f"pmu1{t}", tag="psT")
                pvr1 = psT.tile([1, BT], f32, name=f"pvr1{t}", tag="psT")
                for m in range(KD):
                    pe2b = psA.tile([128, BT], f32, name=f"pe2b_{t}_{m}", tag="psA")
                    for k in range(KD):
                        nc.tensor.matmul(
                            pe2b[:], W21[:, k, bass.ts(m, 128)], g2[:, k, :],
                            start=(k == 0), stop=(k == KD - 1),
                        )
                    sq = sp.tile([128, BT], f32, name=f"sq1{t}_{m}", tag="sq")
                    sqr = sp.tile([128, BT], f32r, name=f"sqr1{t}_{m}", tag="sqr", bufs=1)
                    nc.scalar.activation(
                        sq[:], pe2b[:], AF.Identity,
                        bias=b21[:, m : m + 1], scale=1.0,
                    )
                    nc.vector.tensor_tensor(
                        e11[:, m, :], e11[:, m, :], sq[:], op=OP.add
                    )
                    nc.scalar.activation(sqr[:], e11[:, m, :], AF.Square)
                    s1r = sp.tile([128, BT], f32r, name=f"s1r{t}_{m}", tag="s0r", bufs=1)
                    nc.vector.tensor_copy(s1r[:], e11[:, m, :])
                    nc.tensor.matmul(
                        pmu1[:], ones_r[:], s1r[:],
                        start=(m == 0), stop=(m == KD - 1),
                        skip_group_check=True,
                    )
                    nc.tensor.matmul(
                        pvr1[:], ones_r[:], sqr[:],
                        start=(m == 0), stop=(m == KD - 1),
                        skip_group_check=True,
                    )
                s1 = e11
                if DEBUG_T and t == 0:
                    nc.sync.dma_start(dbgs1_d.ap(), s1[:, 0, :])

                # ---- LN1 scalars ----------------------------------------
                nmu1 = sp.tile([1, BT], f32, name=f"nmu1{t}", tag="nmu1", bufs=1)
                var1 = sp.tile([1, BT], f32, name=f"var1{t}", tag="var1", bufs=1)
                ist1 = sp.tile([1, BT], f32, name=f"ist1{t}", tag="ist1", bufs=1)
                nc.vector.tensor_scalar(nmu1[:], pmu1[:], -1.0 / D, None, op0=OP.mult)
                nc.vector.tensor_tensor(ist1[:], nmu1[:], nmu1[:], op=OP.mult)
                nc.vector.scalar_tensor_tensor(
                    var1[:], pvr1[:], 1.0 / D, ist1[:], op0=OP.mult, op1=OP.subtract
                )
                nc.scalar.activation(var1[:], var1[:], AF.Ln, bias=epsv[:], scale=1.0)
                nc.scalar.activation(ist1[:], var1[:], AF.Exp, bias=0.0, scale=0.5)
                nmu1r = sp.tile([1, BT], f32r, name=f"nmu1r{t}", tag="nmu1r", bufs=1)
                nc.vector.tensor_copy(nmu1r[:], nmu1[:])

                # ---- sims + logits per 128-row subtile -------------------
                for s in range(NSUB):
                    col = t * NSUB + s
                    ssl = bass.ts(s, 128)
                    T = sp.tile([128, C], f32, name=f"T{t}_{s}", tag="T", bufs=1)
                    msk = sp.tile([128, C], f32, name=f"msk{t}_{s}", tag="msk", bufs=1)
                    scr = sp.tile([128, C], f32, name=f"scr{t}_{s}", tag="scr", bufs=1)
                    ssq1 = sp.tile([128, 1], f32, name=f"ssq1{t}_{s}", tag="ssq1")
                    ssq2 = sp.tile([128, 1], f32, name=f"ssq2{t}_{s}", tag="ssq2")
                    aux = sp.tile([128, 1], f32, name=f"aux{t}_{s}", tag="aux")
                    rn = sp.tile([128, 1], f32, name=f"rn{t}_{s}", tag="rn")
                    mrow = sp.tile([128, 1], f32, name=f"mrow{t}_{s}", tag="mrow")
                    sumE = sp.tile([128, 1], f32, name=f"sumE{t}_{s}", tag="sumE")
                    ttar = sp.tile([128, 1], f32, name=f"ttar{t}_{s}", tag="ttar")

                    # S1 = img @ txt (stash into T, sumsq into ssq1)
                    for g, (o, n) in enumerate(CGRPS):
                        p1 = psS.tile([128, n], f32, name=f"p1_{t}_{s}_{g}", tag="psS")
                        for k in range(KD):
                            nc.tensor.matmul(
                                p1[:], imT[:, k, ssl], txt[:, k, o : o + n],
                                start=(k == 0), stop=(k == KD - 1),
                            )
                        nc.scalar.activation(
                            scr[:, o : o + n], p1[:], AF.Square,
                            accum_out=(ssq1[:] if g == 0 else aux[:]),
                        )
                        nc.vector.tensor_copy(T[:, o : o + n], p1[:])
                    nc.vector.tensor_tensor(ssq1[:], ssq1[:], aux[:], op=OP.add)

                    # S2 = s1 @ txw + (-mu1) x ct1 + irstd1 x ct2 (into msk)
                    for g, (o, n) in enumerate(CGRPS):
                        p2 = psS.tile([128, n], f32, name=f"p2_{t}_{s}_{g}", tag="psS")
                        for k in range(KD):
                            nc.tensor.matmul(
                                p2[:], s1[:, k, ssl], txw[:, k, o : o + n],
                                start=(k == 0), stop=False,
                            )
                        nc.tensor.matmul(
                            p2[:], nmu1r[:, ssl], ct1_r[:, o : o + n],
                            start=False, stop=not has_ct2,
                        )
                        if has_ct2:
                            nc.tensor.matmul(
                                p2[:], ist1[:, ssl], ct2[:, o : o + n],
                                start=False, stop=True,
                            )
                        nc.scalar.activation(
                            scr[:, o : o + n], p2[:], AF.Square,
                            accum_out=(ssq2[:] if g == 0 else aux[:]),
                        )
                        nc.vector.tensor_copy(msk[:, o : o + n], p2[:])
                    nc.vector.tensor_tensor(ssq2[:], ssq2[:], aux[:], op=OP.add)
                    if DEBUG_T and t == 0 and s == 0:
                        nc.sync.dma_start(dbgS2_d.ap(), msk[:])
                        nc.sync.dma_start(dbgq1_d.ap(), ssq1[:])
                        nc.sync.dma_start(dbgq2_d.ap(), ssq2[:])

                    # T = S1 * p/sqrt(ssq1) + S2 * (1-p)/sqrt(ssq2)
                    nc.scalar.activation(ssq1[:], ssq1[:], AF.Ln)
                    nc.scalar.activation(rn[:], ssq1[:], AF.Exp, bias=lnp[:], scale=-0.5)
                    if DEBUG_T and t == 0 and s == 0:
                        nc.sync.dma_start(dbgr1_d.ap(), rn[:])
                    nc.vector.tensor_scalar(T[:], T[:], rn[:], None, op0=OP.mult)
                    nc.scalar.activation(ssq2[:], ssq2[:], AF.Ln)
                    nc.scalar.activation(rn[:], ssq2[:], AF.Exp, bias=ln1p[:], scale=-0.5)
                    if DEBUG_T and t == 0 and s == 0:
                        nc.sync.dma_start(dbgr2_d.ap(), rn[:])
                    nc.vector.scalar_tensor_tensor(
                        T[:], msk[:], rn[:], T[:], op0=OP.mult, op1=OP.add
                    )

                    if DEBUG_T and t == 0 and s == 0:
                        nc.sync.dma_start(dbg_d.ap(), T[:])
                    # s' = rL/temp = exp(-0.5 ln(sumsq T) - ln temp)
                    nc.scalar.activation(scr[:], T[:], AF.Square, accum_out=aux[:])
                    nc.scalar.activation(aux[:], aux[:], AF.Ln)
                    nc.scalar.activation(rn[:], aux[:], AF.Exp, bias=nlt[:], scale=-0.5)

                    # max + argmax (first-index tie-break like jnp.argmax)
                    nc.vector.tensor_reduce(mrow[:], T[:], axis=AX.X, op=OP.max)
                    # (T == m) * (iota - BIG); min-reduce -> first argmax
                    nc.vector.scalar_tensor_tensor(
                        scr[:], T[:], mrow[:], iot[:],
                        op0=OP.is_equal, op1=OP.mult,
                    )
                    nc.vector.tensor_reduce(aux[:], scr[:], axis=AX.X, op=OP.min)
                    nc.vector.tensor_scalar(
                        amx_st[:, col : col + 1], aux[:], BIG, None, op0=OP.add
                    )

                    # E = exp((T - m) s'), sumE (fused row-sum)
                    nc.vector.tensor_scalar(
                        aux[:], mrow[:], rn[:], -1.0, op0=OP.mult, op1=OP.mult
                    )
                    nc.scalar.activation(
                        scr[:], T[:], AF.Exp, bias=aux[:], scale=rn[:],
                        accum_out=sumE[:],
                    )
                    # gather T[target]: (iota == tgt) * T, fused row-sum
                    nc.vector.scalar_tensor_tensor(
                        scr[:], iot[:], tgt[:, col : col + 1], T[:],
                        op0=OP.is_equal, op1=OP.mult, accum_out=ttar[:],
                    )
                    # nll = ln(sumE) - (Ttar - m) s'
                    nc.scalar.activation(sumE[:], sumE[:], AF.Ln)
                    nc.vector.tensor_scalar(
                        ttar[:], ttar[:], mrow[:], rn[:],
                        op0=OP.subtract, op1=OP.mult,
                    )
                    nc.vector.tensor_tensor(
                        nll_st[:, col : col + 1], sumE[:], ttar[:], op=OP.subtract
                    )

            nc.sync.dma_start(nll_d.ap(), nll_st[:])
            nc.sync.dma_start(amx_d.ap(), amx_st[:])

    _split_waits(nc)
    return nc


def _pack_lhsT(W):
    """[Kfeat, M] (K-major) -> [128, Kchunks, M]."""
    K, M = W.shape
    return np.ascontiguousarray(
        W.reshape(K // 128, 128, M).transpose(1, 0, 2), dtype=np.float32
    )


def prepare(inputs):
    inp = {k: np.asarray(v) for k, v in inputs.items()}
    x = inp["embeddings"].astype(np.float32)
    img = inp["img_features"].astype(np.float32)
    txtm = inp["txt_features"].astype(np.float32)
    p = np.float64(inp["weight_p"])
    temp = np.float64(inp["temp"])
    tgt = inp["target_ind"].astype(np.int64)
    W0 = inp["W0"].astype(np.float32)
    W2_0 = inp["W2_0"].astype(np.float32)
    W1 = inp["W1"].astype(np.float32)
    W2_1 = inp["W2_1"].astype(np.float32)
    ln0w = inp["ln0_w"].astype(np.float32); ln0b = inp["ln0_b"].astype(np.float32)
    ln1w = inp["ln1_w"].astype(np.float32); ln1b = inp["ln1_b"].astype(np.float32)

    A0 = _pack_lhsT(W0.T)
    W20 = (W2_0 / np.float32(2.0)).T                       # [H(in k), H(out)]
    W20m = np.ascontiguousarray(
        W20.reshape(KH, 128, KH, 128).transpose(1, 2, 0, 3).reshape(128, KH, H),
        dtype=np.float32,
    )  # [p, m_out, k*128+q]
    W1f = _pack_lhsT(ln0w[:, None] * W1.T)
    W21 = _pack_lhsT((W2_1 / np.float32(2.0)).T)
    txt4 = _pack_lhsT(txtm)
    txw4 = _pack_lhsT(ln1w[:, None] * txtm)
    cmu = (ln0w @ W1.T).astype(np.float32)[None, :]
    cb1 = (inp["b1"].astype(np.float32) + ln0b @ W1.T).astype(np.float32)[None, :]
    ct1 = (ln1w @ txtm).astype(np.float32)[None, :]
    ct2 = (ln1b @ txtm).astype(np.float32)[None, :]
    b0c = np.ascontiguousarray(inp["b0"].astype(np.float32).reshape(KH, 128).T)
    b20c = np.ascontiguousarray(inp["b2_0"].astype(np.float32).reshape(KH, 128).T)
    b21c = np.ascontiguousarray(inp["b2_1"].astype(np.float32).reshape(KD, 128).T)
    iota = np.ascontiguousarray(
        np.broadcast_to(
            np.arange(C, dtype=np.float32) - np.float32(BIG), (128, C)
        )
    )
    with np.errstate(divide="ignore"):
        lnp = np.full((128, 1), np.log(p), np.float32)
        ln1p = np.full((128, 1), np.log(1.0 - p), np.float32)
        nlt = np.full((128, 1), -np.log(temp), np.float32)

    shared = dict(
        A0=A0, W20=W20m, W1f=W1f, W21=W21, txt4=txt4, txw4=txw4,
        cmu=cmu, cb1=cb1, ct1=ct1, ct2=ct2, b0c=b0c, b20c=b20c, b21c=b21c,
        iotam=iota, lnp=lnp, ln1p=ln1p, nlt=nlt,
    )
    in_maps = []
    for c in range(NCORES):
        rows = slice(c * ROWS, (c + 1) * ROWS)
        xT = np.ascontiguousarray(x[rows].T).reshape(KD, 128, ROWS)
        imT = np.ascontiguousarray(img[rows].T).reshape(KD, 128, ROWS)
        tgtm = np.ascontiguousarray(
            tgt[rows].reshape(NCOLS, 128).T.astype(np.float32) - np.float32(BIG)
        )
        in_maps.append(dict(shared, xT=xT, imT=imT, tgtm=tgtm))
    has_txw = bool(np.any(ln1w != np.float32(1.0)))
    if not has_txw:
        for m in in_maps:
            m.pop('txw4', None)
    return in_maps, tgt, bool(np.any(cb1)), bool(np.any(ct2)), has_txw


def run(inputs, ntiles=ROWS // BT, prof=False, **kw):
    in_maps, tgt, has_cb1, has_ct2, has_txw = prepare(inputs)
    key = (ntiles, has_cb1, has_ct2, has_txw)
    if key not in _CACHE:
        _CACHE[key] = build(ntiles, has_cb1, has_ct2, has_txw)
    res = run_bass_kernel_spmd(
        _CACHE[key], in_maps, core_ids=list(range(NCORES)), trace=prof, **kw
    )
    nrows = ntiles * BT
    ncols = ntiles * NSUB
    nll_parts, am_parts, tg_parts = [], [], []
    for c in range(NCORES):
        nll_parts.append(res.results[c]["nll"][:, :ncols].T.reshape(-1))
        am_parts.append(res.results[c]["amx"][:, :ncols].T.reshape(-1))
        tg_parts.append(tgt[c * ROWS : c * ROWS + nrows])
    nll_all = np.concatenate(nll_parts)
    am_all = np.concatenate(am_parts).astype(np.int32)
    tgt_used = np.concatenate(tg_parts)
    loss = np.float32(np.mean(nll_all, dtype=np.float64))
    acc = np.int32((am_all == tgt_used).sum())
    return (loss, acc, am_all), res


def kernel(**inputs):
    (loss, acc, am_all), _ = run(inputs)
    return loss, acc, am_all


# revision 15
# speedup vs baseline: 1.0316x; 1.0316x over previous
"""Trainium2 Bass kernel for nn_LLaVA_CLIP (loss_fn).

Math (exact reformulations of the reference):
- row_norm(x * c) == row_norm(x) for positive per-row scale c, so the
  exp(ls_*) factors vanish, sim_* == a_*, out_logits == a_total, and the
  row_norm of desc cancels inside row_norm(desc @ txt).
- quick_gelu(x) = silu(1.702 x)/1.702; silu(z) = 0.5 z (1 + tanh(z/2));
  constants fold into the next matmul's weights (W2/2).
- Both LayerNorms fold into the adjacent matmuls:
    e1_1 = rstd0 (.) [s0 @ (diag(w0ln) W1.T) + (-mu0) x cmu + irstd0 x cb1]
      cmu = w0ln @ W1.T, cb1 = b1 + b0ln @ W1.T, irstd0 = sqrt(var0+eps)
    S2 = s1 @ (diag(w1ln) txt) + (-mu1) x ct1 + irstd1 x ct2
      (the rstd1 row scale cancels in row_norm)
Sharding: batch 65536 -> 8 cores x 8192 rows, weights replicated. MLP runs
feature-major ([feat, batch] tiles); sims/logits run batch-major per 128 rows.
"""
import sys

sys.path.insert(0, "/opt/trn_rl_repo")
if "/root/.axon_site" not in sys.path:
    sys.path.insert(0, "/root/.axon_site")

import numpy as np

import concourse.bass as bass
import concourse.mybir as mybir
from concourse import tile
from concourse.bass_utils import run_bass_kernel_spmd

AF = mybir.ActivationFunctionType
OP = mybir.AluOpType
AX = mybir.AxisListType
f32 = mybir.dt.float32
f32r = mybir.dt.float32r

B, D, H, C = 65536, 512, 1024, 1000
NCORES = 8
ROWS = B // NCORES          # 8192 rows per core
BT = 512                    # batch columns per feature-major tile
NSUB = BT // 128            # batch-major subtiles per tile
KD = D // 128               # 4
KH = H // 128               # 8
NCOLS = ROWS // 128         # 64 staging columns
BIG = float(2 ** 20)
LN_EPS = 1e-5
CGRPS = [(0, 512), (512, 488)]

_CACHE = {}
DEBUG_T = False


def _split_waits(nc, cap=1):
    """This walrus accepts only ~1 sync-wait command per instruction; hoist
    extra waits onto same-engine EventSemaphore carriers (program order on the
    engine preserves semantics)."""
    for fn in nc.m.functions:
        for blk in fn.blocks:
            out = []
            for inst in blk.instructions:
                si = inst.sync_info
                if si is not None and si.on_wait and len(si.on_wait) > cap:
                    waits = list(si.on_wait)
                    keep, extra = waits[:cap], waits[cap:]
                    for i in range(0, len(extra), cap):
                        out.append(
                            mybir.InstEventSemaphore(
                                name=f"waitsplit-{nc.next_id()}",
                                engine=inst.engine,
                                sync_info=mybir.SyncInfo(
                                    on_wait=extra[i : i + cap], on_update=[]
                                ),
                                bass_nofuse=True,
                            )
                        )
                    inst.sync_info = mybir.SyncInfo(
                        on_wait=keep, on_update=list(si.on_update)
                    )
                out.append(inst)
            blk.instructions[:] = out


def build(ntiles, has_cb1, has_ct2, has_txw):
    nc = bass.Bass("TRN2", target_bir_lowering=False, debug=False)
    dt = nc.dram_tensor

    xT_d = dt("xT", [KD, 128, ROWS], f32, kind="ExternalInput")
    imT_d = dt("imT", [KD, 128, ROWS], f32, kind="ExternalInput")
    tgt_d = dt("tgtm", [128, NCOLS], f32, kind="ExternalInput")   # target-BIG
    A0_d = dt("A0", [128, KD, H], f32, kind="ExternalInput")
    W20_d = dt("W20", [128, KH, H], f32, kind="ExternalInput")
    W1f_d = dt("W1f", [128, KH, D], f32, kind="ExternalInput")
    W21_d = dt("W21", [128, KD, D], f32, kind="ExternalInput")
    txt_d = dt("txt4", [128, KD, C], f32, kind="ExternalInput")
    txw_d = dt("txw4", [128, KD, C], f32, kind="ExternalInput") if has_txw else None
    cmu_d = dt("cmu", [1, D], f32, kind="ExternalInput")
    cb1_d = dt("cb1", [1, D], f32, kind="ExternalInput")
    ct1_d = dt("ct1", [1, C], f32, kind="ExternalInput")
    ct2_d = dt("ct2", [1, C], f32, kind="ExternalInput")
    b0_d = dt("b0c", [128, KH], f32, kind="ExternalInput")
    b20_d = dt("b20c", [128, KH], f32, kind="ExternalInput")
    b21_d = dt("b21c", [128, KD], f32, kind="ExternalInput")
    iota_d = dt("iotam", [128, C], f32, kind="ExternalInput")     # iota-BIG
    lnp_d = dt("lnp", [128, 1], f32, kind="ExternalInput")        # ln(p)
    ln1p_d = dt("ln1p", [128, 1], f32, kind="ExternalInput")      # ln(1-p)
    nlt_d = dt("nlt", [128, 1], f32, kind="ExternalInput")        # -ln(temp)
    nll_d = dt("nll", [128, NCOLS], f32, kind="ExternalOutput")
    amx_d = dt("amx", [128, NCOLS], f32, kind="ExternalOutput")
    dbg_d = dt("dbgT", [128, C], f32, kind="ExternalOutput") if DEBUG_T else None
    if DEBUG_T:
        dbge1_d = dt("dbge1", [128, BT], f32, kind="ExternalOutput")
        dbgg_d = dt("dbgg", [128, BT], f32, kind="ExternalOutput")
        dbgs0_d = dt("dbgs0", [128, BT], f32, kind="ExternalOutput")
        dbge11_d = dt("dbge11", [128, BT], f32, kind="ExternalOutput")
        dbgs1_d = dt("dbgs1", [128, BT], f32, kind="ExternalOutput")
        dbgS2_d = dt("dbgS2", [128, C], f32, kind="ExternalOutput")
        dbgq1_d = dt("dbgq1", [128, 1], f32, kind="ExternalOutput")
        dbgq2_d = dt("dbgq2", [128, 1], f32, kind="ExternalOutput")
        dbgr1_d = dt("dbgr1", [128, 1], f32, kind="ExternalOutput")
        dbgr2_d = dt("dbgr2", [128, 1], f32, kind="ExternalOutput")

    with tile.TileContext(nc) as tc:
        with (
            tc.tile_pool(name="wpool", bufs=1) as wp,
            tc.tile_pool(name="w20ring", bufs=3) as wr,
            tc.tile_pool(name="inpool", bufs=2) as ip,
            tc.tile_pool(name="actpool", bufs=1) as ap_,
            tc.tile_pool(name="scpool", bufs=2) as sp,
            tc.tile_pool(name="psA", bufs=3, space="PSUM") as psA,
            tc.tile_pool(name="psStat", bufs=2, space="PSUM") as psT,
            tc.tile_pool(name="psSim", bufs=3, space="PSUM") as psS,
        ):
            # ---- resident constants -----------------------------------
            A0 = wp.tile([128, KD, H], f32, name="A0s")
            W1f = wp.tile([128, KH, D], f32, name="W1fs")
            W21 = wp.tile([128, KD, D], f32, name="W21s")
            txt = wp.tile([128, KD, C], f32, name="txts")
            txw = wp.tile([128, KD, C], f32, name="txws") if has_txw else txt
            cmu = wp.tile([1, D], f32, name="cmus")
            cb1 = wp.tile([1, D], f32, name="cb1s")
            ct1 = wp.tile([1, C], f32, name="ct1s")
            ct2 = wp.tile([1, C], f32, name="ct2s")
            b0 = wp.tile([128, KH], f32, name="b0s")
            b20 = wp.tile([128, KH], f32, name="b20s")
            b21 = wp.tile([128, KD], f32, name="b21s")
            iot = wp.tile([128, C], f32, name="iots")
            lnp = wp.tile([128, 1], f32, name="lnps")
            ln1p = wp.tile([128, 1], f32, name="ln1ps")
            nlt = wp.tile([128, 1], f32, name="nlts")
            tgt = wp.tile([128, NCOLS], f32, name="tgts")
            ones = wp.tile([128, 1], f32, name="oness")
            epsv = wp.tile([1, 1], f32, name="epsvs")
            onesr = wp.tile([1, 128], f32, name="onesrs")
            nll_st = wp.tile([128, NCOLS], f32, name="nllst")
            amx_st = wp.tile([128, NCOLS], f32, name="amxst")

            for t_, d_ in [
                (A0, A0_d), (W1f, W1f_d), (W21, W21_d), (txt, txt_d),
                *([(txw, txw_d)] if has_txw else []),
                (cmu, cmu_d), (cb1, cb1_d), (ct1, ct1_d),
                (ct2, ct2_d), (b0, b0_d), (b20, b20_d), (b21, b21_d),
                (iot, iota_d), (lnp, lnp_d), (ln1p, ln1p_d), (nlt, nlt_d),
                (tgt, tgt_d),
            ]:
                nc.sync.dma_start(t_[:], d_.ap())
            nc.gpsimd.memset(ones[:], 1.0)
            nc.gpsimd.memset(epsv[:], float(LN_EPS))
            nc.gpsimd.memset(onesr[:], 1.0)
            ones_r = wp.tile([128, 1], f32r, name="onesrr")
            onesr_r = wp.tile([1, 128], f32r, name="onesrrr")
            cmu_r = wp.tile([1, D], f32r, name="cmurr")
            ct1_r = wp.tile([1, C], f32r, name="ct1rr")
            nc.vector.tensor_copy(ones_r[:], ones[:])
            nc.vector.tensor_copy(onesr_r[:], onesr[:])
            nc.vector.tensor_copy(cmu_r[:], cmu[:])
            nc.vector.tensor_copy(ct1_r[:], ct1[:])

            for t in range(ntiles):
                bsl = bass.ts(t, BT)

                xT = ip.tile([128, KD, BT], f32, name=f"xT{t}", tag="xT")
                imT = ip.tile([128, KD, BT], f32, name=f"imT{t}", tag="imT")
                for k in range(KD):
                    nc.sync.dma_start(xT[:, k, :], xT_d.ap()[k, :, bsl])
                    nc.sync.dma_start(imT[:, k, :], imT_d.ap()[k, :, bsl])

                # ---- L0a: e1 = W0 @ xT + b0; g' = e1 (1 + tanh(.851 e1))
                e1s = ap_.tile([128, KH, BT], f32, name=f"e1s{t}", tag="e1s")
                gs = ap_.tile([128, KH, BT], f32, name=f"gs{t}", tag="gs")
                for m in range(KH):
                    pe1 = psA.tile([128, BT], f32, name=f"pe1_{t}_{m}", tag="psA")
                    for k in range(KD):
                        nc.tensor.matmul(
                            pe1[:], A0[:, k, bass.ts(m, 128)], xT[:, k, :],
                            start=(k == 0), stop=(k == KD - 1),
                        )
                    nc.scalar.activation(
                        e1s[:, m, :], pe1[:], AF.Identity,
                        bias=b0[:, m : m + 1], scale=1.0,
                    )
                    th = sp.tile([128, BT], f32, name=f"th{t}_{m}", tag="th")
                    nc.scalar.activation(
                        th[:], e1s[:, m, :], AF.Tanh, bias=0.0, scale=0.851
                    )
                    # g' = (th + 1) * e1
                    nc.vector.scalar_tensor_tensor(
                        gs[:, m, :], th[:], 1.0, e1s[:, m, :],
                        op0=OP.add, op1=OP.mult,
                    )

                if DEBUG_T and t == 0:
                    nc.sync.dma_start(dbge1_d.ap(), e1s[:, 0, :])
                    nc.sync.dma_start(dbgg_d.ap(), gs[:, 0, :])
                # ---- L0b: e2 = (W2_0/2) @ g'; s0 = e1+e2 (into e1s); stats
                pmu0 = psT.tile([1, BT], f32, name=f"pmu0{t}", tag="psT")
                pvr0 = psT.tile([1, BT], f32, name=f"pvr0{t}", tag="psT")
                for m in range(KH):
                    w20c = wr.tile([128, H], f32, name=f"w20_{t}_{m}", tag="w20")
                    nc.sync.dma_start(w20c[:], W20_d.ap()[:, m, :])
                    pe2 = psA.tile([128, BT], f32, name=f"pe2_{t}_{m}", tag="psA")
                    for k in range(KH):
                        nc.tensor.matmul(
                            pe2[:], w20c[:, bass.ts(k, 128)], gs[:, k, :],
                            start=(k == 0), stop=(k == KH - 1),
                        )
                    sq = sp.tile([128, BT], f32, name=f"sq0{t}_{m}", tag="sq")
                    sqr = sp.tile([128, BT], f32r, name=f"sqr0{t}_{m}", tag="sqr", bufs=1)
                    nc.scalar.activation(
                        sq[:], pe2[:], AF.Identity,
                        bias=b20[:, m : m + 1], scale=1.0,
                    )
                    # s0 overwrites e1s (e1 is dead after this)
                    nc.vector.tensor_tensor(
                        e1s[:, m, :], e1s[:, m, :], sq[:], op=OP.add
                    )
                    nc.scalar.activation(sqr[:], e1s[:, m, :], AF.Square)
                    nc.tensor.matmul(
                        pmu0[:], ones[:], e1s[:, m, :],
                        start=(m == 0), stop=(m == KH - 1),
                        skip_group_check=True,
                    )
                    nc.tensor.matmul(
                        pvr0[:], ones_r[:], sqr[:],
                        start=(m == 0), stop=(m == KH - 1),
                        skip_group_check=True,
                    )
                s0 = e1s
                if DEBUG_T and t == 0:
                    nc.sync.dma_start(dbgs0_d.ap(), s0[:, 0, :])

                # ---- LN0 scalars on [1, BT] ------------------------------
                nmu0 = sp.tile([1, BT], f32, name=f"nmu0{t}", tag="nmu0", bufs=1)
                var0 = sp.tile([1, BT], f32, name=f"var0{t}", tag="var0", bufs=1)
                rst0 = sp.tile([1, BT], f32, name=f"rst0{t}", tag="rst0", bufs=1)
                ist0 = sp.tile([1, BT], f32, name=f"ist0{t}", tag="ist0", bufs=1)
                nc.vector.tensor_scalar(nmu0[:], pmu0[:], -1.0 / H, None, op0=OP.mult)
                # var = msq - mu^2 = (pvr0/H) - nmu0^2
                nc.vector.tensor_tensor(rst0[:], nmu0[:], nmu0[:], op=OP.mult)
                nc.vector.scalar_tensor_tensor(
                    var0[:], pvr0[:], 1.0 / H, rst0[:], op0=OP.mult, op1=OP.subtract
                )
                nc.scalar.activation(var0[:], var0[:], AF.Ln, bias=epsv[:], scale=1.0)
                nc.scalar.activation(rst0[:], var0[:], AF.Exp, bias=0.0, scale=-0.5)
                nc.scalar.activation(ist0[:], var0[:], AF.Exp, bias=0.0, scale=0.5)
                rst0r = sp.tile([1, BT], f32r, name=f"rst0r{t}", tag="rst0r", bufs=1)
                nmu0r = sp.tile([1, BT], f32r, name=f"nmu0r{t}", tag="nmu0r", bufs=1)
                nc.vector.tensor_copy(rst0r[:], rst0[:])
                nc.vector.tensor_copy(nmu0r[:], nmu0[:])

                # ---- L1a: P = s0@W1f + (-mu0)x cmu + irstd0 x cb1;
                #      e1_1 = rstd0 (.) P; g2' = e1_1(1+tanh(.851 e1_1)) ----
                e11 = ap_.tile([128, KD, BT], f32, name=f"e11{t}", tag="e11")
                g2 = ap_.tile([128, KD, BT], f32, name=f"g2{t}", tag="gs")
                prb = psA.tile([128, BT], f32, name=f"prb{t}", tag="psA")
                nc.tensor.matmul(prb[:], onesr_r[:], rst0r[:], start=True, stop=True)
                rstb = sp.tile([128, BT], f32, name=f"rstb{t}", tag="rstb", bufs=1)
                nc.scalar.copy(rstb[:], prb[:])
                for m in range(KD):
                    pp = psA.tile([128, BT], f32, name=f"pp_{t}_{m}", tag="psA")
                    for k in range(KH):
                        nc.tensor.matmul(
                            pp[:], W1f[:, k, bass.ts(m, 128)], s0[:, k, :],
                            start=(k == 0), stop=False,
                        )
                    nc.tensor.matmul(
                        pp[:], cmu_r[:, bass.ts(m, 128)], nmu0r[:],
                        start=False, stop=not has_cb1,
                    )
                    if has_cb1:
                        nc.tensor.matmul(
                            pp[:], cb1[:, bass.ts(m, 128)], ist0[:],
                            start=False, stop=True,
                        )
                    nc.vector.tensor_tensor(e11[:, m, :], pp[:], rstb[:], op=OP.mult)
                    th2 = sp.tile([128, BT], f32, name=f"th2{t}_{m}", tag="th")
                    nc.scalar.activation(
                        th2[:], e11[:, m, :], AF.Tanh, bias=0.0, scale=0.851
                    )
                    nc.vector.scalar_tensor_tensor(
                        g2[:, m, :], th2[:], 1.0, e11[:, m, :],
                        op0=OP.add, op1=OP.mult,
                    )

                if DEBUG_T and t == 0:
                    nc.sync.dma_start(dbge11_d.ap(), e11[:, 0, :])
                # ---- L1b: e2_1 = (W2_1/2) @ g2'; s1 = e1_1+e2_1; stats ---
                pmu1 = psT.tile([1, BT], f32, name=f"pmu1{t}", tag="psT")
                pvr1 = psT.tile([1, BT], f32, name=f"pvr1{t}", tag="psT")
                for m in range(KD):
                    pe2b = psA.tile([128, BT], f32, name=f"pe2b_{t}_{m}", tag="psA")
                    for k in range(KD):
                        nc.tensor.matmul(
                            pe2b[:], W21[:, k, bass.ts(m, 128)], g2[:, k, :],
                            start=(k == 0), stop=(k == KD - 1),
                        )
                    sq = sp.tile([128, BT], f32, name=f"sq1{t}_{m}", tag="sq")
                    sqr = sp.tile([128, BT], f32r, name=f"sqr1{t}_{m}", tag="sqr", bufs=1)
                    nc.scalar.activation(
                        sq[:], pe2b[:], AF.Identity,
                        bias=b21[:, m : m + 1], scale=1.0,
                    )
                    nc.vector.tensor_tensor(
                        e11[:, m, :], e11[:, m, :], sq[:], op=OP.add
                    )
                    nc.scalar.activation(sqr[:], e11[:, m, :], AF.Square)
                    nc.tensor.matmul(
                        pmu1[:], ones[:], e11[:, m, :],
                        start=(m == 0), stop=(m == KD - 1),
                        skip_group_check=True,
                    )
                    nc.tensor.matmul(
                        pvr1[:], ones_r[:], sqr[:],
                        start=(m == 0), stop=(m == KD - 1),
                        skip_group_check=True,
                    )
                s1 = e11
                if DEBUG_T and t == 0:
                    nc.sync.dma_start(dbgs1_d.ap(), s1[:, 0, :])

                # ---- LN1 scalars ----------------------------------------
                nmu1 = sp.tile([1, BT], f32, name=f"nmu1{t}", tag="nmu1", bufs=1)
                var1 = sp.tile([1, BT], f32, name=f"var1{t}", tag="var1", bufs=1)
                ist1 = sp.tile([1, BT], f32, name=f"ist1{t}", tag="ist1", bufs=1)
                nc.vector.tensor_scalar(nmu1[:], pmu1[:], -1.0 / D, None, op0=OP.mult)
                nc.vector.tensor_tensor(ist1[:], nmu1[:], nmu1[:], op=OP.mult)
                nc.vector.scalar_tensor_tensor(
                    var1[:], pvr1[:], 1.0 / D, ist1[:], op0=OP.mult, op1=OP.subtract
                )
                nc.scalar.activation(var1[:], var1[:], AF.Ln, bias=epsv[:], scale=1.0)
                nc.scalar.activation(ist1[:], var1[:], AF.Exp, bias=0.0, scale=0.5)
                nmu1r = sp.tile([1, BT], f32r, name=f"nmu1r{t}", tag="nmu1r", bufs=1)
                nc.vector.tensor_copy(nmu1r[:], nmu1[:])

                # ---- sims + logits per 128-row subtile -------------------
                for s in range(NSUB):
                    col = t * NSUB + s
                    ssl = bass.ts(s, 128)
                    T = sp.tile([128, C], f32, name=f"T{t}_{s}", tag="T", bufs=1)
                    msk = sp.tile([128, C], f32, name=f"msk{t}_{s}", tag="msk", bufs=1)
                    scr = sp.tile([128, C], f32, name=f"scr{t}_{s}", tag="scr", bufs=1)
                    ssq1 = sp.tile([128, 1], f32, name=f"ssq1{t}_{s}", tag="ssq1")
                    ssq2 = sp.tile([128, 1], f32, name=f"ssq2{t}_{s}", tag="ssq2")
                    aux = sp.tile([128, 1], f32, name=f"aux{t}_{s}", tag="aux")
                    rn = sp.tile([128, 1], f32, name=f"rn{t}_{s}", tag="rn")
                    mrow = sp.tile([128, 1], f32, name=f"mrow{t}_{s}", tag="mrow")
                    sumE = sp.tile([128, 1], f32, name=f"sumE{t}_{s}", tag="sumE")
                    ttar = sp.tile([128, 1], f32, name=f"ttar{t}_{s}", tag="ttar")

                    # S1 = img @ txt (stash into T, sumsq into ssq1)
                    for g, (o, n) in enumerate(CGRPS):
                        p1 = psS.tile([128, n], f32, name=f"p1_{t}_{s}_{g}", tag="psS")
                        for k in range(KD):
                            nc.tensor.matmul(
                                p1[:], imT[:, k, ssl], txt[:, k, o : o + n],
                                start=(k == 0), stop=(k == KD - 1),
                            )
                        nc.scalar.activation(
                            scr[:, o : o + n], p1[:], AF.Square,
                            accum_out=(ssq1[:] if g == 0 else aux[:]),
                        )
                        nc.vector.tensor_copy(T[:, o : o + n], p1[:])
                    nc.vector.tensor_tensor(ssq1[:], ssq1[:], aux[:], op=OP.add)

                    # S2 = s1 @ txw + (-mu1) x ct1 + irstd1 x ct2 (into msk)
                    for g, (o, n) in enumerate(CGRPS):
                        p2 = psS.tile([128, n], f32, name=f"p2_{t}_{s}_{g}", tag="psS")
                        for k in range(KD):
                            nc.tensor.matmul(
                                p2[:], s1[:, k, ssl], txw[:, k, o : o + n],
                                start=(k == 0), stop=False,
                            )
                        nc.tensor.matmul(
                            p2[:], nmu1r[:, ssl], ct1_r[:, o : o + n],
                            start=False, stop=not has_ct2,
                        )
                        if has_ct2:
                            nc.tensor.matmul(
                                p2[:], ist1[:, ssl], ct2[:, o : o + n],
                                start=False, stop=True,
                            )
                        nc.scalar.activation(
                            scr[:, o : o + n], p2[:], AF.Square,
                            accum_out=(ssq2[:] if g == 0 else aux[:]),
                        )
                        nc.vector.tensor_copy(msk[:, o : o + n], p2[:])
                    nc.vector.tensor_tensor(ssq2[:], ssq2[:], aux[:], op=OP.add)
                    if DEBUG_T and t == 0 and s == 0:
                        nc.sync.dma_start(dbgS2_d.ap(), msk[:])
                        nc.sync.dma_start(dbgq1_d.ap(), ssq1[:])
                        nc.sync.dma_start(dbgq2_d.ap(), ssq2[:])

                    # T = S1 * p/sqrt(ssq1) + S2 * (1-p)/sqrt(ssq2)
                    nc.scalar.activation(ssq1[:], ssq1[:], AF.Ln)
                    nc.scalar.activation(rn[:], ssq1[:], AF.Exp, bias=lnp[:], scale=-0.5)
                    if DEBUG_T and t == 0 and s == 0:
                        nc.sync.dma_start(dbgr1_d.ap(), rn[:])
                    nc.vector.tensor_scalar(T[:], T[:], rn[:], None, op0=OP.mult)
                    nc.scalar.activation(ssq2[:], ssq2[:], AF.Ln)
                    nc.scalar.activation(rn[:], ssq2[:], AF.Exp, bias=ln1p[:], scale=-0.5)
                    if DEBUG_T and t == 0 and s == 0:
                        nc.sync.dma_start(dbgr2_d.ap(), rn[:])
                    nc.vector.scalar_tensor_tensor(
                        T[:], msk[:], rn[:], T[:], op0=OP.mult, op1=OP.add
                    )

                    if DEBUG_T and t == 0 and s == 0:
                        nc.sync.dma_start(dbg_d.ap(), T[:])
                    # s' = rL/temp = exp(-0.5 ln(sumsq T) - ln temp)
                    nc.scalar.activation(scr[:], T[:], AF.Square, accum_out=aux[:])
                    nc.scalar.activation(aux[:], aux[:], AF.Ln)
                    nc.scalar.activation(rn[:], aux[:], AF.Exp, bias=nlt[:], scale=-0.5)

                    # max + argmax (first-index tie-break like jnp.argmax)
                    nc.vector.tensor_reduce(mrow[:], T[:], axis=AX.X, op=OP.max)
                    # (T == m) * (iota - BIG); min-reduce -> first argmax
                    nc.vector.scalar_tensor_tensor(
                        scr[:], T[:], mrow[:], iot[:],
                        op0=OP.is_equal, op1=OP.mult,
                    )
                    nc.vector.tensor_reduce(aux[:], scr[:], axis=AX.X, op=OP.min)
                    nc.vector.tensor_scalar(
                        amx_st[:, col : col + 1], aux[:], BIG, None, op0=OP.add
                    )

                    # E = exp((T - m) s'), sumE (fused row-sum)
                    nc.vector.tensor_scalar(
                        aux[:], mrow[:], rn[:], -1.0, op0=OP.mult, op1=OP.mult
                    )
                    nc.scalar.activation(
                        scr[:], T[:], AF.Exp, bias=aux[:], scale=rn[:],
                        accum_out=sumE[:],
                    )
                    # gather T[target]: (iota == tgt) * T, fused row-sum
                    nc.vector.scalar_tensor_tensor(
                        scr[:], iot[:], tgt[:, col : col + 1], T[:],
                        op0=OP.is_equal, op1=OP.mult, accum_out=ttar[:],
                    )
                    # nll = ln(sumE) - (Ttar - m) s'
                    nc.scalar.activation(sumE[:], sumE[:], AF.Ln)
                    nc.vector.tensor_scalar(
                        ttar[:], ttar[:], mrow[:], rn[:],
                        op0=OP.subtract, op1=OP.mult,
                    )
                    nc.vector.tensor_tensor(
                        nll_st[:, col : col + 1], sumE[:], ttar[:], op=OP.subtract
                    )

            nc.sync.dma_start(nll_d.ap(), nll_st[:])
            nc.sync.dma_start(amx_d.ap(), amx_st[:])

    _split_waits(nc)
    return nc


def _pack_lhsT(W):
    """[Kfeat, M] (K-major) -> [128, Kchunks, M]."""
    K, M = W.shape
    return np.ascontiguousarray(
        W.reshape(K // 128, 128, M).transpose(1, 0, 2), dtype=np.float32
    )


def prepare(inputs):
    inp = {k: np.asarray(v) for k, v in inputs.items()}
    x = inp["embeddings"].astype(np.float32)
    img = inp["img_features"].astype(np.float32)
    txtm = inp["txt_features"].astype(np.float32)
    p = np.float64(inp["weight_p"])
    temp = np.float64(inp["temp"])
    tgt = inp["target_ind"].astype(np.int64)
    W0 = inp["W0"].astype(np.float32)
    W2_0 = inp["W2_0"].astype(np.float32)
    W1 = inp["W1"].astype(np.float32)
    W2_1 = inp["W2_1"].astype(np.float32)
    ln0w = inp["ln0_w"].astype(np.float32); ln0b = inp["ln0_b"].astype(np.float32)
    ln1w = inp["ln1_w"].astype(np.float32); ln1b = inp["ln1_b"].astype(np.float32)

    A0 = _pack_lhsT(W0.T)
    W20 = (W2_0 / np.float32(2.0)).T                       # [H(in k), H(out)]
    W20m = np.ascontiguousarray(
        W20.reshape(KH, 128, KH, 128).transpose(1, 2, 0, 3).reshape(128, KH, H),
        dtype=np.float32,
    )  # [p, m_out, k*128+q]
    W1f = _pack_lhsT(ln0w[:, None] * W1.T)
    W21 = _pack_lhsT((W2_1 / np.float32(2.0)).T)
    txt4 = _pack_lhsT(txtm)
    txw4 = _pack_lhsT(ln1w[:, None] * txtm)
    cmu = (ln0w @ W1.T).astype(np.float32)[None, :]
    cb1 = (inp["b1"].astype(np.float32) + ln0b @ W1.T).astype(np.float32)[None, :]
    ct1 = (ln1w @ txtm).astype(np.float32)[None, :]
    ct2 = (ln1b @ txtm).astype(np.float32)[None, :]
    b0c = np.ascontiguousarray(inp["b0"].astype(np.float32).reshape(KH, 128).T)
    b20c = np.ascontiguousarray(inp["b2_0"].astype(np.float32).reshape(KH, 128).T)
    b21c = np.ascontiguousarray(inp["b2_1"].astype(np.float32).reshape(KD, 128).T)
    iota = np.ascontiguousarray(
        np.broadcast_to(
            np.arange(C, dtype=np.float32) - np.float32(BIG), (128, C)
        )
    )
    with np.errstate(divide="ignore"):
        lnp = np.full((128, 1), np.log(p), np.float32)
        ln1p = np.full((128, 1), np.log(1.0 - p), np.float32)
        nlt = np.full((128, 1), -np.log(temp), np.float32)

    shared = dict(
        A0=A0, W20=W20m, W1f=W1f, W21=W21, txt4=txt4, txw4=txw4,
        cmu=cmu, cb1=cb1, ct1=ct1, ct2=ct2, b0c=b0c, b20c=b20c, b21c=b21c,
        iotam=iota, lnp=lnp, ln1p=ln1p, nlt=nlt,
    )
    in_maps = []
    for c in range(NCORES):
        rows = slice(c * ROWS, (c + 1) * ROWS)
        xT = np.ascontiguousarray(x[rows].T).reshape(KD, 128, ROWS)
        imT = np.ascontiguousarray(img[rows].T).reshape(KD, 128, ROWS)
        tgtm = np.ascontiguousarray(
            tgt[rows].reshape(NCOLS, 128).T.astype(np.float32) - np.float32(BIG)
        )
        in_maps.append(dict(shared, xT=xT, imT=imT, tgtm=tgtm))
    has_txw = bool(np.any(ln1w != np.float32(1.0)))
    if not has_txw:
        for m in in_maps:
            m.pop('txw4', None)
    return in_maps, tgt, bool(np.any(cb1)), bool(np.any(ct2)), has_txw


def run(inputs, ntiles=ROWS // BT, prof=False, **kw):
    in_maps, tgt, has_cb1, has_ct2, has_txw = prepare(inputs)
    key = (ntiles, has_cb1, has_ct2, has_txw)
    if key not in _CACHE:
        _CACHE[key] = build(ntiles, has_cb1, has_ct2, has_txw)
    res = run_bass_kernel_spmd(
        _CACHE[key], in_maps, core_ids=list(range(NCORES)), trace=prof, **kw
    )
    nrows = ntiles * BT
    ncols = ntiles * NSUB
    nll_parts, am_parts, tg_parts = [], [], []
    for c in range(NCORES):
        nll_parts.append(res.results[c]["nll"][:, :ncols].T.reshape(-1))
        am_parts.append(res.results[c]["amx"][:, :ncols].T.reshape(-1))
        tg_parts.append(tgt[c * ROWS : c * ROWS + nrows])
    nll_all = np.concatenate(nll_parts)
    am_all = np.concatenate(am_parts).astype(np.int32)
    tgt_used = np.concatenate(tg_parts)
    loss = np.float32(np.mean(nll_all, dtype=np.float64))
    acc = np.int32((am_all == tgt_used).sum())
    return (loss, acc, am_all), res


def kernel(**inputs):
    (loss, acc, am_all), _ = run(inputs)
    return loss, acc, am_all
